# revision 7
# baseline (speedup 1.0000x reference)
# ContextRCNN attention-bias kernel for 8 Trainium2 NeuronCores.
#
# Reference computation:
#   central:[N,C,7,7] -> mean-pool -> Q-MLP -> l2norm -> queries [N,QK]
#   context:[T,C,7,7] -> mean-pool -> K/V-MLPs (K l2normed)
#   softmax(Q @ K^T * 6.25) @ V -> final MLP -> [N, C]
#
# Distribution (one SPMD NEFF on 8 cores, rank == q-shard == t-shard index):
#   - central rows sharded N/8=512 per core; context rows sharded T/8=1536,
#     so the 822MB of input is read exactly once across the chip (~103MB/core,
#     the memory roofline: ~288us/core at fair-share HBM bandwidth).
#   - Each core pools+MLPs its own shards, computes local queries, AllGathers
#     the query matrix (bf16), then computes the partial attention of ALL
#     4096 queries against its LOCAL keys/values: numer_r = exp(S_r)^T @
#     [V_r | 1]. A ReduceScatter sums the partials over cores and hands rank
#     r exactly its q-shard rows; divide by the gathered denominator column
#     and run the final MLP locally.
#   - Softmax needs no max-subtraction: logits are 6.25*cosine in [-6.25,6.25].
#
# Optimization history (trace-driven; baseline 610us):
#   v2 (511us): all matmul operands bf16 (single-pass PE at full rate, half
#     LDWEIGHTS/AG payload; fp32 ran 2-pass LOW_HIGH). raw_pool bufs 2->6 so
#     the pooling DMA->reduce pipeline runs at HBM bandwidth instead of a
#     6us/tile semaphore-latency cadence. DVE work (transpose copy-outs, V
#     copies) moved to the scalar engine; DVE must stay under the DMA floor.
#   v3: the Q->AllGather chain is split in two (per 2 q-tiles) so the AG
#     latency overlaps the central stream and attention can start ~60us
#     earlier. numerator accumulators and the ReduceScatter payload are fp16
#     with a 1/64 prescale folded into vw2/ones (host-emulated rel err 4e-3,
#     range margin 148x): halves RS bytes and DVE add traffic. The last
#     context chunk is split into two 1-tile chunks so the post-stream
#     attention tail halves. Final MLP runs per-128-row-tile, pipelined with
#     the rs_out loads. rs_in stores split across the sync+gpsimd queues.

import numpy as np
from contextlib import ExitStack

import ml_dtypes

import concourse.bass as bass
import concourse.mybir as mybir
import concourse.tile as tile
from concourse import bacc
from concourse.bass_utils import run_bass_kernel_spmd
from concourse.masks import make_identity

AF = mybir.ActivationFunctionType
DT = mybir.dt.float32
BF = mybir.dt.bfloat16
F16 = mybir.dt.float16

M = 8                    # cores
N, T, C, S = 4096, 12288, 256, 7
NS, TS = N // M, T // M  # 512 q rows / 1536 kv rows per core
H = 512                  # MLP hidden
D = 256                  # QK == VD == C
SS = S * S               # 49
SCALE = 1.0 / (0.01 * C ** 0.5)   # 6.25
PRESCALE = 64.0          # numerator prescale (folded into vw2 + ones)
NT_Q = NS // 128         # 4  q-tiles per core
NT_T = TS // 128         # 12 t-tiles per core
NCOL = D + 2             # V plus ones cols
CSPLIT = 4               # channel split for the raw pooling loads
CCH = C // CSPLIT        # channels per load
RAWF = CCH * SS          # floats per partition row per load
CHUNKS = [(0, 2), (2, 2), (4, 2), (6, 2), (8, 2), (10, 1), (11, 1)]
RAW_BUFS = 6


def build_nc(mode="bf16"):
    nc = bacc.Bacc("TRN2", target_bir_lowering=False, debug=False, num_devices=M)
    if mode == "bf16":
        adt = mdt = BF
        rdt = F16
    elif mode == "fp32r":
        adt = mybir.dt.float32r
        mdt = DT
        rdt = DT
    else:
        adt = mdt = DT
        rdt = DT

    central = nc.dram_tensor("central_sh", [NS, C, S, S], DT, kind="ExternalInput")
    context = nc.dram_tensor("context_sh", [TS, C, S, S], DT, kind="ExternalInput")
    wnames = ["qw1", "qw2", "kw1", "kw2", "vw1", "vw2", "fw1", "fw2"]
    wshapes = {"1": [C, H], "2": [H, D]}
    wdram = {n: nc.dram_tensor(n, wshapes[n[-1]], mdt, kind="ExternalInput")
             for n in wnames}
    out_sh = nc.dram_tensor("out_sh", [NS, C], DT, kind="ExternalOutput")

    # split AllGather: one per pair of q-tiles so AG latency hides under the
    # central stream
    qt_in = [nc.dram_tensor(f"qt_in{p}", [D, 256], adt) for p in range(2)]
    qt_out = [nc.dram_tensor(f"qt_out{p}", [M * D, 256], adt,
                             addr_space="Shared") for p in range(2)]
    rs_in = nc.dram_tensor("rs_in", [N, NCOL], rdt)
    rs_out = nc.dram_tensor("rs_out", [NS, NCOL], rdt)

    with tile.TileContext(nc) as tc, ExitStack() as ctx:
        ident_pool = ctx.enter_context(tc.tile_pool(name="ident", bufs=1))
        ident_f = ident_pool.tile([128, 128], DT, tag="idf", name="idf")
        make_identity(nc, ident_f[:])

        # SBUF tensors that live across phases
        kvq_pool = ctx.enter_context(tc.tile_pool(name="kvq", bufs=1))
        ones_col = kvq_pool.tile([128, 2], adt, tag="ones", name="ones")
        nc.gpsimd.memset(ones_col[:], (1.0 / PRESCALE) if mode == "bf16" else 1.0)
        kt_sb = [kvq_pool.tile([128, TS], adt, tag=f"kt{i}", name=f"kt{i}")
                 for i in range(2)]
        vo_sb = [kvq_pool.tile([128, NCOL], adt, tag=f"vo{i}", name=f"vo{i}")
                 for i in range(NT_T)]
        qt_all = [kvq_pool.tile([128, NS], adt, tag=f"qta{i}", name=f"qta{i}")
                  for i in range(2 * M)]
        # numerator accumulators for all 32 q-tiles (summed over chunks)
        nm_pool = ctx.enter_context(tc.tile_pool(name="nm", bufs=1))
        nm_sb = [nm_pool.tile([128, NCOL], rdt, tag=f"nm{i}", name=f"nm{i}")
                 for i in range(N // 128)]

        def transpose128(dst_sb, src_sb, tp_pool):
            """dst = src[128,128]^T via PE, fp32 in psum; scalar copy-out
            casts to dst's dtype (bf16 for the matmul pipeline)."""
            ps = tp_pool.tile([128, 256], DT, tag="ps_small", name="tp")
            nc.tensor.transpose(ps[:, 0:128], src_sb, ident_f[:])
            nc.scalar.copy(dst_sb, ps[:, 0:128])

        def load_w(pool, name):
            shape = wshapes[name[-1]]
            tiles = []
            for i in range(shape[0] // 128):
                t = pool.tile([128, shape[1]], mdt, tag=f"{name}_{i}",
                              name=f"{name}_{i}")
                nc.sync.dma_start(t[:], wdram[name].ap()[i * 128:(i + 1) * 128, :])
                tiles.append(t)
            return tiles

        def pool_rows(dram_t, row0, raw_pool, pooled_pool):
            """Sum-pool 128 rows of [rows,C,7,7] -> pooled [128, C] tile.
            (The 1/49 mean scale cancels in l2norm for Q/K and is folded
            into vw2 on the host for V.)"""
            pooled = pooled_pool.tile([128, C], DT)
            src = dram_t.ap().rearrange("t c h w -> t (c h w)")
            for cs in range(CSPLIT):
                raw = raw_pool.tile([128, RAWF], DT, tag="raw")
                nc.sync.dma_start(
                    raw[:], src[row0:row0 + 128, cs * RAWF:(cs + 1) * RAWF])
                nc.vector.reduce_sum(
                    pooled[:, cs * CCH:(cs + 1) * CCH],
                    raw[:].rearrange("p (c s) -> p c s", s=SS),
                    axis=mybir.AxisListType.X)
            return pooled

        def mlp_l1_T(w1_tiles, xT, xcol0, out_tiles, ocol0, nfree, ps_pool):
            """hidden^T[h, ocol0:ocol0+nfree] = relu(w1^T @ x^T[:, xcol0:])"""
            for ht in range(H // 128):
                for c0 in range(0, nfree, 512):
                    w = min(512, nfree - c0)
                    ps = ps_pool.tile([128, 512], DT, tag="mm512")
                    for ck in range(C // 128):
                        nc.tensor.matmul(
                            ps[:, 0:w],
                            w1_tiles[ck][:, ht * 128:(ht + 1) * 128],
                            xT[ck][:, xcol0 + c0:xcol0 + c0 + w],
                            start=(ck == 0), stop=(ck == 1))
                    nc.scalar.activation(
                        out_tiles[ht][:, ocol0 + c0:ocol0 + c0 + w],
                        ps[:, 0:w], AF.Relu)

        def mlp_l2_nat(hid_tiles, w2_tiles, nt, ps_pool):
            """x[n,d] psum tile = hidden @ w2 for 128-row block nt."""
            ps = ps_pool.tile([128, 256], DT, tag="ps_small", name="l2ps")
            for hk in range(H // 128):
                nc.tensor.matmul(
                    ps[:],
                    hid_tiles[hk][:, nt * 128:(nt + 1) * 128],
                    w2_tiles[hk][:],
                    start=(hk == 0), stop=(hk == 3))
            return ps

        def l2norm_recip(src_ps, pool):
            """1/||row|| as [128,1] from psum tile."""
            sq = pool.tile([128, D], DT, tag="sq", name="sq")
            nc.scalar.activation(sq[:], src_ps[:], AF.Square)
            ssq = pool.tile([128, 1], DT, tag="ssq", name="ssq")
            nc.vector.reduce_sum(ssq[:], sq[:], axis=mybir.AxisListType.X)
            nrm = pool.tile([128, 1], DT, tag="nrm", name="nrm")
            nc.scalar.activation(nrm[:], ssq[:], AF.Sqrt)
            rcp = pool.tile([128, 1], DT, tag="rcp", name="rcp")
            nc.vector.reciprocal(rcp[:], nrm[:])
            return rcp

        with tc.tile_pool(name="raw", bufs=RAW_BUFS) as raw_pool, \
             tc.tile_pool(name="pooled", bufs=3) as pooled_pool, \
             tc.tile_pool(name="ptq", bufs=1) as ptq_pool, \
             tc.tile_pool(name="ptc", bufs=2) as ptc_pool, \
             tc.tile_pool(name="wA", bufs=1) as wA_pool, \
             tc.tile_pool(name="hid", bufs=1) as hid_pool, \
             tc.tile_pool(name="small", bufs=2) as small_pool, \
             tc.tile_pool(name="est", bufs=6) as e_pool, \
             tc.tile_pool(name="ps512", bufs=2, space="PSUM") as ps512, \
             tc.tile_pool(name="psSm", bufs=2, space="PSUM") as psSm, \
             tc.tile_pool(name="psST", bufs=2, space="PSUM") as ps_st, \
             tc.tile_pool(name="psNM", bufs=2, space="PSUM") as ps_nm:

            qw1 = load_w(wA_pool, "qw1"); qw2 = load_w(wA_pool, "qw2")
            kw1 = load_w(wA_pool, "kw1"); kw2 = load_w(wA_pool, "kw2")
            vw1 = load_w(wA_pool, "vw1"); vw2 = load_w(wA_pool, "vw2")

            # --- central: pool -> Q MLP -> l2norm*6.25 -> AllGather, one
            # pair of 128-row q-tiles at a time so AG0 fires mid-stream ---
            cenT = [ptq_pool.tile([128, NS], mdt, tag=f"cenT{i}", name=f"cenT{i}")
                    for i in range(2)]
            hq = [hid_pool.tile([128, NS], mdt, tag=f"hid{i}", name=f"hq{i}")
                  for i in range(4)]
            qt_sh = [wA_pool.tile([128, NS], adt, tag=f"qtsh{i}", name=f"qtsh{i}")
                     for i in range(2)]
            for pair in range(2):
                for nt in (2 * pair, 2 * pair + 1):
                    pooled = pool_rows(central, nt * 128, raw_pool, pooled_pool)
                    for ct in range(2):
                        transpose128(cenT[ct][:, nt * 128:(nt + 1) * 128],
                                     pooled[:, ct * 128:(ct + 1) * 128], psSm)
                mlp_l1_T(qw1, cenT, pair * 256, hq, pair * 256, 256, ps512)
                for nt in (2 * pair, 2 * pair + 1):
                    q_ps = mlp_l2_nat(hq, qw2, nt, psSm)
                    rcp = l2norm_recip(q_ps, small_pool)
                    qn = small_pool.tile([128, D], DT, tag="qn", name="qn")
                    nc.vector.tensor_scalar(
                        qn[:], q_ps[:], rcp[:], SCALE,
                        op0=mybir.AluOpType.mult, op1=mybir.AluOpType.mult)
                    for ct in range(2):
                        transpose128(qt_sh[ct][:, nt * 128:(nt + 1) * 128],
                                     qn[:, ct * 128:(ct + 1) * 128], psSm)
                for ct in range(2):
                    nc.gpsimd.dma_start(
                        qt_in[pair].ap()[ct * 128:(ct + 1) * 128, :],
                        qt_sh[ct][:, pair * 256:(pair + 1) * 256])
                nc.gpsimd.collective_compute(
                    "AllGather", mybir.AluOpType.bypass,
                    replica_groups=[list(range(M))],
                    ins=[qt_in[pair][:]], outs=[qt_out[pair][:]])
                for r in range(M):
                    for dk in range(2):
                        nc.gpsimd.dma_start(
                            qt_all[2 * r + dk][:, pair * 256:(pair + 1) * 256],
                            qt_out[pair].ap()[r * D + dk * 128:
                                              r * D + (dk + 1) * 128, :])

            # --- context chunks: pool -> K/V MLP -> attention, pipelined ---
            def kv_mlp(ci):
                tt0, sz = CHUNKS[ci]
                ctxT = [ptc_pool.tile([128, 256], mdt, tag=f"ctxT{i}",
                                      name=f"ctxT{ci}_{i}") for i in range(2)]
                for j in range(sz):
                    pooled = pool_rows(context, (tt0 + j) * 128,
                                       raw_pool, pooled_pool)
                    for ct in range(2):
                        transpose128(ctxT[ct][:, j * 128:(j + 1) * 128],
                                     pooled[:, ct * 128:(ct + 1) * 128],
                                     psSm)
                # K path for this chunk
                hk = [hid_pool.tile([128, 256], mdt, tag=f"hid{i}",
                                    name=f"hk{ci}_{i}") for i in range(4)]
                mlp_l1_T(kw1, ctxT, 0, hk, 0, sz * 128, ps512)
                for j in range(sz):
                    k_ps = mlp_l2_nat(hk, kw2, j, psSm)
                    rcp = l2norm_recip(k_ps, small_pool)
                    kn = small_pool.tile([128, D], DT, tag="qn", name="kn")
                    nc.vector.tensor_scalar_mul(kn[:], k_ps[:], rcp[:])
                    for ct in range(2):
                        tcol = (tt0 + j) * 128
                        transpose128(kt_sb[ct][:, tcol:tcol + 128],
                                     kn[:, ct * 128:(ct + 1) * 128],
                                     psSm)
                # V path for this chunk
                hv = [hid_pool.tile([128, 256], mdt, tag=f"hid{i}",
                                    name=f"hv{ci}_{i}") for i in range(4)]
                mlp_l1_T(vw1, ctxT, 0, hv, 0, sz * 128, ps512)
                for j in range(sz):
                    v_ps = mlp_l2_nat(hv, vw2, j, psSm)
                    nc.scalar.copy(vo_sb[tt0 + j][:, 0:D], v_ps[:])
                    nc.scalar.copy(vo_sb[tt0 + j][:, D:NCOL], ones_col[:])

            def attention(ci):
                # attention of ALL q-blocks against this chunk's K/V.
                # Emit q-block (qb) score+exp before the previous q-block's
                # numer matmuls so exp latency hides under PE work.
                tt0, sz = CHUNKS[ci]
                prev_e = None

                def numer_for(e_tiles, qb):
                    for qt in range(NT_Q):
                        nm_ps = ps_nm.tile([128, NCOL], DT, tag="nmps",
                                           name="nmps")
                        for j in range(sz):
                            nc.tensor.matmul(
                                nm_ps[:],
                                e_tiles[j][:, qt * 128:(qt + 1) * 128],
                                vo_sb[tt0 + j][:],
                                start=(j == 0), stop=(j == sz - 1),
                                skip_group_check=True)
                        qrow = qb * NT_Q + qt
                        if ci == 0:
                            nc.scalar.copy(nm_sb[qrow][:], nm_ps[:])
                        else:
                            nc.vector.tensor_add(nm_sb[qrow][:],
                                                 nm_sb[qrow][:], nm_ps[:])

                for qb in range(M):
                    e_tiles = []
                    for j in range(sz):
                        st = ps_st.tile([128, 512], DT, tag="st")
                        for dk in range(2):
                            tcol = (tt0 + j) * 128
                            nc.tensor.matmul(
                                st[:], kt_sb[dk][:, tcol:tcol + 128],
                                qt_all[2 * qb + dk][:],
                                start=(dk == 0), stop=(dk == 1))
                        e_sb = e_pool.tile([128, 512], adt, tag="e", name="esb")
                        nc.scalar.activation(e_sb[:], st[:], AF.Exp)
                        e_tiles.append(e_sb)
                    if prev_e is not None:
                        numer_for(prev_e, qb - 1)
                    prev_e = e_tiles
                numer_for(prev_e, M - 1)

            for ci in range(len(CHUNKS)):
                kv_mlp(ci)
                attention(ci)

            # rs_in stores: split across sync+gpsimd queues so the issue
            # serialization after the last chunk halves
            for qrow in range(N // 128):
                eng = nc.sync if qrow % 2 == 0 else nc.gpsimd
                eng.dma_start(rs_in.ap()[qrow * 128:(qrow + 1) * 128, :],
                              nm_sb[qrow][:])

        nc.gpsimd.collective_compute(
            "ReduceScatter", mybir.AluOpType.add,
            replica_groups=[list(range(M))],
            ins=[rs_in[:]], outs=[rs_out[:]])

        # ---------------- divide + final MLP (per-tile pipelined) ----------
        with tc.tile_pool(name="fin", bufs=2) as fin_pool, \
             tc.tile_pool(name="wC", bufs=1) as wC_pool, \
             tc.tile_pool(name="hidC", bufs=1) as hidC_pool, \
             tc.tile_pool(name="psC", bufs=2, space="PSUM") as psC, \
             tc.tile_pool(name="psTC", bufs=2, space="PSUM") as psTC:
            fw1 = load_w(wC_pool, "fw1"); fw2 = load_w(wC_pool, "fw2")
            attnT = [fin_pool.tile([128, NS], mdt, tag=f"attnT{i}",
                                   name=f"attnT{i}") for i in range(2)]
            hf = [hidC_pool.tile([128, NS], mdt, tag=f"hidC{i}", name=f"hfC{i}")
                  for i in range(4)]
            for ntile in range(NT_Q):
                att = fin_pool.tile([128, NCOL], rdt, tag="att", name="att")
                nc.sync.dma_start(
                    att[:], rs_out.ap()[ntile * 128:(ntile + 1) * 128, :])
                att32 = fin_pool.tile([128, NCOL], DT, tag="att32", name="at32")
                nc.scalar.copy(att32[:], att[:])
                rd = fin_pool.tile([128, 1], DT, tag="rd", name="rd")
                nc.vector.reciprocal(rd[:], att32[:, D:D + 1])
                an = fin_pool.tile([128, D], DT, tag="an", name="an")
                nc.vector.tensor_scalar_mul(an[:], att32[:, 0:D], rd[:])
                for ct in range(2):
                    transpose128(attnT[ct][:, ntile * 128:(ntile + 1) * 128],
                                 an[:, ct * 128:(ct + 1) * 128], psTC)
                mlp_l1_T(fw1, attnT, ntile * 128, hf, ntile * 128, 128, psC)
                o_ps = psC.tile([128, 256], DT, tag="ops", name="ops")
                for hk_i in range(H // 128):
                    nc.tensor.matmul(
                        o_ps[:], hf[hk_i][:, ntile * 128:(ntile + 1) * 128],
                        fw2[hk_i][:], start=(hk_i == 0), stop=(hk_i == 3))
                o_sb = fin_pool.tile([128, D], DT, tag="osb", name="osb")
                nc.vector.tensor_copy(o_sb[:], o_ps[:])
                nc.sync.dma_start(
                    out_sh.ap()[ntile * 128:(ntile + 1) * 128, :], o_sb[:])

    nc.finalize()
    return nc


_NC_CACHE = {}


def kernel(central_features, context_features, qw1, qw2, kw1, kw2,
           vw1, vw2, fw1, fw2, _trace=False, _return_results=False,
           _mode="bf16"):
    if _mode not in _NC_CACHE:
        _NC_CACHE[_mode] = build_nc(mode=_mode)
    nc = _NC_CACHE[_mode]

    wdt = ml_dtypes.bfloat16 if _mode == "bf16" else np.float32
    vscale = float(SS) * (PRESCALE if _mode == "bf16" else 1.0)
    weights = {"qw1": qw1, "qw2": qw2, "kw1": kw1, "kw2": kw2,
               "vw1": vw1, "vw2": np.asarray(vw2, np.float32) / vscale,
               "fw1": fw1, "fw2": fw2}
    weights = {k: np.ascontiguousarray(np.asarray(v, np.float32).astype(wdt))
               for k, v in weights.items()}
    central_features = np.ascontiguousarray(central_features, dtype=np.float32)
    context_features = np.ascontiguousarray(context_features, dtype=np.float32)

    in_maps = []
    for r in range(M):
        m = {"central_sh": central_features[r * NS:(r + 1) * NS],
             "context_sh": context_features[r * TS:(r + 1) * TS]}
        m.update(weights)
        in_maps.append(m)

    res = run_bass_kernel_spmd(nc, in_maps, core_ids=list(range(M)),
                               trace=_trace)
    out = np.concatenate([res.results[r]["out_sh"] for r in range(M)], axis=0)
    if _return_results:
        return out, res
    return out


if __name__ == "__main__":
    rng = np.random.default_rng(0)
    f = lambda *s: rng.standard_normal(s, dtype=np.float32)
    ins = dict(central_features=f(N, C, S, S), context_features=f(T, C, S, S),
               qw1=f(C, H) * 0.02, qw2=f(H, D) * 0.02,
               kw1=f(C, H) * 0.02, kw2=f(H, D) * 0.02,
               vw1=f(C, H) * 0.02, vw2=f(H, D) * 0.02,
               fw1=f(D, H) * 0.02, fw2=f(H, C) * 0.02)
    out = kernel(**ins)
    print(out.shape, out.dtype, np.abs(out).max())
